# revision 1
# baseline (speedup 1.0000x reference)
"""Trainium2 Bass kernel for nn_DynamicSphericalTorch (GNN message passing).

Computation (per batch b of 64, N=16384 nodes, D=32 edges/node, 6 steps):
    s = tanh(x * w_in + biases[0])
    for t in 0..5:  s = tanh(sum_d s[edge_src[t,:,d]] * edge_w[t,:,d] + biases[t+1])

Sharding: batch dim (64) split across 8 NeuronCores (8 batches/core), all
edge/weight/bias data replicated per core -> zero cross-core communication.

Per-core layout:
  - SBUF state table [128 part, 16385] f32: partition p holds the FULL state
    of batch p%8 (16-way replicated), plus a constant-1.0 slot at col 16384
    used to fold the per-step bias into the gather/MAC (bias = weight of a
    fake 33rd edge whose source is the constant slot).
  - GpSimd ap_gather: the 8 Q7 cores each own 16 partitions and gather an
    independent 1/8 node-chunk of the edge list (indices host-prepacked
    int16, wrapped mod 16 across each group's partitions).
  - DVE: elementwise multiply by edge weights + segment-reduce over the 33
    slots per node.  ACT: tanh.  DMA: weight streaming + broadcast of new
    states back into the replicated table.
"""

import os
import sys

import numpy as np

for _p in ("/opt/trn_rl_repo",):
    if os.path.isdir(_p) and _p not in sys.path:
        sys.path.insert(0, _p)

B, N, D, STEPS = 64, 16384, 32, 6
NCORES = 8
BLOC = B // NCORES  # 8 batches per core
G = 8  # gather groups (= Q7 cores), 16 partitions each
NPG = N // G  # 2048 nodes per group
TS_N = 128  # nodes per group per tile
TILES = NPG // TS_N  # 16
DD = D + 1  # 32 edges + 1 bias slot
ET = TS_N * DD  # 4224 indices per group per tile
ISLOT = ET // 16  # 264 idx slots per partition
TAB = N + 1  # state table length (const-1.0 at col N)

_CACHE = {}


def _build_kernel():
    from contextlib import ExitStack

    import concourse.bass as bass
    import concourse.tile as tile
    from concourse import bacc, mybir

    f32 = mybir.dt.float32
    i16 = mybir.dt.int16

    nc = bacc.Bacc("TRN2", target_bir_lowering=False, debug=False,
                   num_devices=NCORES)

    idx_ap = nc.dram_tensor("idx", [STEPS, TILES, 128, ISLOT], i16,
                            kind="ExternalInput").ap()
    w_ap = nc.dram_tensor("w", [STEPS, TILES, G, ET], f32,
                          kind="ExternalInput").ap()
    xl_ap = nc.dram_tensor("xl", [128, NPG], f32, kind="ExternalInput").ap()
    winl_ap = nc.dram_tensor("winl", [G, NPG], f32, kind="ExternalInput").ap()
    b0l_ap = nc.dram_tensor("b0l", [G, NPG], f32, kind="ExternalInput").ap()
    out_ap = nc.dram_tensor("out", [BLOC, N], f32, kind="ExternalOutput").ap()

    with tile.TileContext(nc) as tc, ExitStack() as ctx:
        cpool = ctx.enter_context(tc.tile_pool(name="const", bufs=1))
        gpool = ctx.enter_context(tc.tile_pool(name="gath", bufs=3))
        wpool = ctx.enter_context(tc.tile_pool(name="wts", bufs=3))
        ipool = ctx.enter_context(tc.tile_pool(name="idxs", bufs=3))
        apool = ctx.enter_context(tc.tile_pool(name="agg", bufs=4))

        state = cpool.tile([128, TAB], f32, tag="state")
        stag = cpool.tile([128, NPG], f32, tag="stag")

        nc.vector.memset(state[:, N:N + 1], 1.0)

        def redistribute():
            # stag[16g:16g+8, :] holds new states of (batches 0..7, nodes of
            # group g); broadcast into every table row of the right batch.
            for gs in range(G):
                src = stag[16 * gs:16 * gs + BLOC, :]
                cols = slice(gs * NPG, (gs + 1) * NPG)
                for gd in range(G):
                    for r in range(2):
                        dst = state[16 * gd + 8 * r:16 * gd + 8 * r + BLOC,
                                    cols]
                        nc.sync.dma_start(dst, src)

        # ---- input layer: s0 = tanh(x * w_in + biases[0]) ----
        xt = gpool.tile([128, NPG], f32, tag="gt")
        nc.sync.dma_start(xt[:], xl_ap[:])
        wt0 = wpool.tile([128, NPG], f32, tag="wt")
        nc.scalar.dma_start(wt0[:], winl_ap.unsqueeze(1)
                            .broadcast_to([G, 16, NPG]))
        bt0 = wpool.tile([128, NPG], f32, tag="wt")
        nc.scalar.dma_start(bt0[:], b0l_ap.unsqueeze(1)
                            .broadcast_to([G, 16, NPG]))
        nc.vector.tensor_mul(xt[:], xt[:], wt0[:])
        nc.vector.tensor_add(xt[:], xt[:], bt0[:])
        nc.scalar.activation(stag[:], xt[:],
                             mybir.ActivationFunctionType.Tanh)
        redistribute()

        # ---- message-passing steps ----
        for t in range(STEPS):
            for k in range(TILES):
                it = ipool.tile([128, ISLOT], i16, tag="it")
                nc.sync.dma_start(it[:], idx_ap[t, k])

                gt = gpool.tile([128, ET], f32, tag="gt")
                nc.gpsimd.ap_gather(gt[:], state[:], it[:], channels=128,
                                    num_elems=TAB, d=1, num_idxs=ET)

                wt = wpool.tile([128, ET], f32, tag="wt")
                nc.scalar.dma_start(wt[:], w_ap[t, k].unsqueeze(1)
                                    .broadcast_to([G, 16, ET]))

                nc.vector.tensor_mul(gt[:], gt[:], wt[:])
                at = apool.tile([128, TS_N], f32, tag="at")
                nc.vector.tensor_reduce(
                    at[:], gt[:].rearrange("p (n d) -> p n d", d=DD),
                    axis=mybir.AxisListType.X, op=mybir.AluOpType.add)
                nc.scalar.activation(stag[:, k * TS_N:(k + 1) * TS_N], at[:],
                                     mybir.ActivationFunctionType.Tanh)
            if t < STEPS - 1:
                redistribute()

        # ---- output: stag holds final states ----
        for gs in range(G):
            nc.sync.dma_start(out_ap[:, gs * NPG:(gs + 1) * NPG],
                              stag[16 * gs:16 * gs + BLOC, :])

    nc.compile()
    return nc


def _prep_static(edge_src, edge_w, biases, w_in):
    # indices with the bias slot appended (points at the const-1.0 entry)
    idx33 = np.empty((STEPS, N, DD), np.int16)
    idx33[:, :, :D] = edge_src.astype(np.int16)
    idx33[:, :, D] = N
    # [t, n, dd] -> [t, g, k, islot, j] -> [t, k, 128, islot]  (wrapped mod 16)
    a = idx33.reshape(STEPS, G, TILES, ISLOT, 16)
    idx_host = np.ascontiguousarray(
        a.transpose(0, 2, 1, 4, 3).reshape(STEPS, TILES, 128, ISLOT))

    w33 = np.empty((STEPS, N, DD), np.float32)
    w33[:, :, :D] = edge_w
    w33[:, :, D] = biases[1:]
    w_host = np.ascontiguousarray(
        w33.reshape(STEPS, G, TILES, ET).transpose(0, 2, 1, 3))

    winl = np.ascontiguousarray(w_in.reshape(G, NPG).astype(np.float32))
    b0l = np.ascontiguousarray(biases[0].reshape(G, NPG).astype(np.float32))
    return idx_host, w_host, winl, b0l


def _prep_x(x, core):
    xc = x[BLOC * core:BLOC * (core + 1)]  # [8, N]
    a = xc.reshape(BLOC, G, NPG).transpose(1, 0, 2)  # [g, b, c]
    return np.ascontiguousarray(
        np.concatenate([a, a], axis=1).reshape(128, NPG))


def kernel(x, w_in, edge_w, biases, edge_src):
    from concourse.bass_utils import run_bass_kernel_spmd

    if "nc" not in _CACHE:
        _CACHE["nc"] = _build_kernel()
    nc = _CACHE["nc"]

    x = np.asarray(x, np.float32)
    idx_host, w_host, winl, b0l = _prep_static(
        np.asarray(edge_src), np.asarray(edge_w, np.float32),
        np.asarray(biases, np.float32), np.asarray(w_in, np.float32))

    in_maps = []
    for c in range(NCORES):
        in_maps.append({
            "idx": idx_host,
            "w": w_host,
            "xl": _prep_x(x, c),
            "winl": winl,
            "b0l": b0l,
        })
    res = run_bass_kernel_spmd(nc, in_maps, list(range(NCORES)))
    out = np.concatenate([res.results[c]["out"] for c in range(NCORES)],
                         axis=0)
    return out.astype(np.float32)



# revision 6
# speedup vs baseline: 4.9976x; 4.9976x over previous
"""Trainium2 Bass kernel for nn_DynamicSphericalTorch (GNN message passing).

Computation (B=64 batches, N=16384 nodes, D=32 edges/node, 6 steps):
    s = tanh(x * w_in + biases[0])
    for t in 0..5:  s = tanh(sum_e s[edge_src[t,:,e]] * edge_w[t,:,e] + b[t+1])

Design notes
------------
The dominant hardware cost is the GpSimd gather (~30ns per index column on
TRN2: SBUF read commands do not pipeline), so the layout maximizes useful
values per gather index:

  - Nodes are sharded across the 8 NeuronCores (2048 nodes/device); every
    device keeps the states of ALL 64 batches packed 4 batches per partition
    ("d=4").  One ap_gather index then yields 16 partitions x 4 = 64 batch
    values, so each (node, edge) pair of the whole problem is gathered
    exactly once per step.
  - The recurrence amplifies per-step noise ~17x, so fp16 states fail the
    2e-2 gate (measured 0.6).  States are stored as uint16 Q15 fixed point
    (tanh outputs live in [-1,1]): q = rtn(s*32767 + 32768).  That keeps the
    2-byte gather while cutting quantization error 16x vs fp16.  Weights are
    pre-scaled by 1/32767 on the host and the +32768 offset is folded into
    the bias: sum_e q*w' - 32768*sum_e w' = sum_e s*w.  All multiply /
    reduce arithmetic runs in f32 on the DVE.
  - State table per partition: [16384 nodes x 4 batches] uint16 (128 KB).
    Partition 16q+j holds batches {4j..4j+3}; Q7 core q gathers the edges of
    its own node sub-chunk.  The table is addressed by a permuted slot id
    (slot order [g(4), r(8), q'(8), nl(64)]) so the per-step state
    rebroadcast is expressible as 3-dim DMAs.
  - Per step: 16 gather tiles (512 idx) -> f32 multiply (DVE, weights
    broadcast-DMAed from HBM) -> tree reduction over the 32 edges (f32) ->
    +bias, tanh, Q15 quantize (ACT).  New states go through an 8-rank
    AllGather (DRAM bounce, 4 groups/step) and are re-broadcast into every
    device's table, pipelined under the gathers of the same step.
"""

import os
import sys

import numpy as np

for _p in ("/opt/trn_rl_repo",):
    if os.path.isdir(_p) and _p not in sys.path:
        sys.path.insert(0, _p)

B, N, D, STEPS = 64, 16384, 32, 6
NCORES = 8          # devices
ND = N // NCORES    # nodes per device (2048)
NPC = ND // 8       # nodes per Q7 core (256)
TILES = 16          # gather tiles per step
NPT = NPC // TILES  # nodes per core per tile (16)
IPT = NPT * D       # gather indices per core per tile (512)
ISLOT = IPT // 16   # idx slots per partition (32)
DP = 4              # batches packed per partition
NE_DECL = 16376     # declared num_elems (fits the 2**15-word field);
                    # real idx go to 16383 and the ucode ignores num_elems
GROUPS = 4          # AllGather / rebroadcast groups per step
TPG = TILES // GROUPS   # tiles per group (4)
CPG = TPG * NPT * DP    # snew cols per group (256)
QSCALE = 32767.0
QOFF = 32768.0

_CACHE = {}


def _build_kernel():
    from contextlib import ExitStack

    import concourse.bass as bass
    import concourse.tile as tile
    from concourse import bacc, mybir

    f32 = mybir.dt.float32
    u16 = mybir.dt.uint16
    i16 = mybir.dt.int16

    nc = bacc.Bacc("TRN2", target_bir_lowering=False, debug=False,
                   num_devices=NCORES)

    idx_ap = nc.dram_tensor("idx", [STEPS, TILES, 128, ISLOT], i16,
                            kind="ExternalInput").ap()
    w_ap = nc.dram_tensor("w", [STEPS, TILES, 8, IPT * DP], f32,
                          kind="ExternalInput").ap()
    bias_ap = nc.dram_tensor("bias", [STEPS, 128, NPC * DP], f32,
                             kind="ExternalInput").ap()
    xin_ap = nc.dram_tensor("xin", [128, NPC * DP], f32,
                            kind="ExternalInput").ap()
    winp_ap = nc.dram_tensor("winp", [128, NPC * DP], f32,
                             kind="ExternalInput").ap()
    b0p_ap = nc.dram_tensor("b0p", [128, NPC * DP], f32,
                            kind="ExternalInput").ap()
    out_ap = nc.dram_tensor("out", [128, NPC * DP], f32,
                            kind="ExternalOutput").ap()

    with tile.TileContext(nc) as tc, ExitStack() as ctx:
        cpool = ctx.enter_context(tc.tile_pool(name="const", bufs=1))
        ipool = ctx.enter_context(tc.tile_pool(name="idxs", bufs=3))
        wpool = ctx.enter_context(tc.tile_pool(name="wts", bufs=2))
        gpool = ctx.enter_context(tc.tile_pool(name="gath", bufs=2))
        ppool = ctx.enter_context(tc.tile_pool(name="prod", bufs=2))
        spool = ctx.enter_context(tc.tile_pool(name="snew", bufs=2))
        bpool = ctx.enter_context(tc.tile_pool(name="bias", bufs=2))
        tpool = ctx.enter_context(tc.tile_pool(name="tanh", bufs=2))
        dram = ctx.enter_context(tc.tile_pool(name="dram", bufs=2,
                                              space="DRAM"))

        table = cpool.tile([128, N * DP], u16, tag="table")

        def exchange(snew):
            """AllGather snew [128, NPC*DP] u16; rebroadcast into table."""
            for g in range(GROUPS):
                inb = dram.tile([16, 8 * CPG], u16, tag=f"inb{g}")
                outb = dram.tile([128, 8 * CPG], u16, tag=f"outb{g}")
                # snew[(16q'+j), g*CPG + c] -> inb[j, q'*CPG + c]
                for qp in range(8):
                    eng = nc.sync if qp % 2 == 0 else nc.scalar
                    eng.dma_start(
                        inb[:, qp * CPG:(qp + 1) * CPG],
                        snew[16 * qp:16 * (qp + 1),
                             g * CPG:(g + 1) * CPG])
                nc.gpsimd.collective_compute(
                    "AllGather", mybir.AluOpType.bypass,
                    replica_groups=[list(range(NCORES))],
                    ins=[inb[:].opt()], outs=[outb[:].opt()])
                # outb[16r+j, (q' nl kk)] -> table[16q+j (all q),
                #   g*16384 + r*2048 + (q' nl kk)]
                sv = outb[:].rearrange("(r j) c -> j r c", j=16)
                for q in range(8):
                    eng = nc.sync if q % 2 == 0 else nc.scalar
                    eng.dma_start(
                        table[16 * q:16 * (q + 1),
                              g * 16384:(g + 1) * 16384]
                        .rearrange("j (r c) -> j r c", r=8),
                        sv)

        def quantize(dst_u16, src_f32):
            nc.scalar.activation(dst_u16, src_f32,
                                 mybir.ActivationFunctionType.Copy,
                                 bias=QOFF, scale=QSCALE)

        # ---- input layer: s0 = tanh(x * w_in + biases[0]) ----
        xt = cpool.tile([128, NPC * DP], f32, tag="xt")
        wt0 = cpool.tile([128, NPC * DP], f32, tag="wt0")
        bt0 = cpool.tile([128, NPC * DP], f32, tag="bt0")
        nc.sync.dma_start(xt[:], xin_ap)
        nc.sync.dma_start(wt0[:], winp_ap)
        nc.sync.dma_start(bt0[:], b0p_ap)
        nc.vector.tensor_mul(xt[:], xt[:], wt0[:])
        nc.vector.tensor_add(xt[:], xt[:], bt0[:])
        t0 = tpool.tile([128, NPC * DP], f32, tag="tt0")
        nc.scalar.activation(t0[:], xt[:],
                             mybir.ActivationFunctionType.Tanh)
        s0 = spool.tile([128, NPC * DP], u16, tag="snew")
        quantize(s0[:], t0[:])
        exchange(s0)

        # ---- message-passing steps ----
        for t in range(STEPS):
            last = t == STEPS - 1
            snew = spool.tile([128, NPC * DP], u16, tag="snew")
            bt = bpool.tile([128, NPC * DP], f32, tag="bt")
            nc.sync.dma_start(bt[:], bias_ap[t])
            if last:
                fout = cpool.tile([128, NPC * DP], f32, tag="fout")
            for k in range(TILES):
                it = ipool.tile([128, ISLOT], i16, tag="it")
                nc.sync.dma_start(it[:], idx_ap[t, k])
                wt = wpool.tile([128, IPT * DP], f32, tag="wt")
                nc.scalar.dma_start(wt[:], w_ap[t, k].unsqueeze(1)
                                    .broadcast_to([8, 16, IPT * DP]))

                gt = gpool.tile([128, IPT * DP], u16, tag="gt")
                nc.gpsimd.ap_gather(gt[:], table[:, :NE_DECL * DP], it[:],
                                    channels=128, num_elems=NE_DECL, d=DP,
                                    num_idxs=IPT)

                pt = ppool.tile([128, IPT * DP], f32, tag="pt")
                nc.vector.tensor_mul(pt[:], gt[:], wt[:])
                # tree reduce over e=32 within [p, n(NPT), e(32), b(4)]
                pv = pt[:].rearrange("p (n e b) -> p n e b", e=D, b=DP)
                half = D // 2
                while half >= 1:
                    nc.vector.tensor_add(pv[:, :, :half, :],
                                         pv[:, :, :half, :],
                                         pv[:, :, half:2 * half, :])
                    half //= 2
                cols = slice(k * NPT * DP, (k + 1) * NPT * DP)
                agg = pv[:, :, 0, :]  # [p, NPT, 4], node stride D*DP
                nc.vector.tensor_add(agg, agg, bt[:, cols]
                                     .rearrange("p (n b) -> p n b", b=DP))
                if last:
                    nc.scalar.activation(
                        fout[:, cols].rearrange("p (n b) -> p n b", b=DP),
                        agg, mybir.ActivationFunctionType.Tanh)
                else:
                    tt = tpool.tile([128, NPT * DP], f32, tag="tt")
                    nc.scalar.activation(
                        tt[:].rearrange("p (n b) -> p n b", b=DP),
                        agg, mybir.ActivationFunctionType.Tanh)
                    quantize(snew[:, cols], tt[:])
            if not last:
                exchange(snew)

        nc.sync.dma_start(out_ap, fout[:])

    nc.compile()
    return nc


def _prep_static(edge_src, edge_w, biases, w_in, x):
    """Host-side packing. Returns per-device input maps."""
    edge_src = np.asarray(edge_src)
    edge_w = np.asarray(edge_w, np.float32)
    biases = np.asarray(biases, np.float32)
    w_in = np.asarray(w_in, np.float32)
    x = np.asarray(x, np.float32)

    # node id -> table slot, slot order [g(4), r(8), q'(8), nl(64)]
    m = np.arange(N)
    sigma = ((m % 256) // 64 * 4096 + (m // 2048) * 512
             + (m % 2048) // 256 * 64 + m % 64)
    slot_src = sigma[edge_src]

    wq = edge_w * np.float32(1.0 / QSCALE)          # [t, n, e]
    bq = biases[1:] - np.float32(QOFF) * wq.sum(axis=2)  # [t, n]

    in_maps = []
    for c in range(NCORES):
        lo = ND * c
        # [t, node(2048), e] for this device, grouped [t, q, k, npt, e]
        src_c = slot_src[:, lo:lo + ND, :].reshape(STEPS, 8, TILES, NPT, D)
        w_c = wq[:, lo:lo + ND, :].reshape(STEPS, 8, TILES, NPT, D)

        # idx: [t, k, 128, ISLOT]; partition 16q+p gets L[q, s*16+p]
        L = src_c.transpose(0, 2, 1, 3, 4).reshape(STEPS, TILES, 8, IPT)
        idx = np.ascontiguousarray(
            L.reshape(STEPS, TILES, 8, ISLOT, 16).transpose(0, 1, 2, 4, 3)
            .reshape(STEPS, TILES, 128, ISLOT)).astype(np.int16)

        # w: [t, k, 8, IPT*DP] f32: per-core values repeated over the
        # 4 batch slots (the 16-partition replication happens on-device
        # via a broadcast DMA)
        Wf = w_c.transpose(0, 2, 1, 3, 4).reshape(STEPS, TILES, 8, IPT)
        w_host = np.ascontiguousarray(
            np.repeat(Wf[..., None], DP, axis=-1)
            .reshape(STEPS, TILES, 8, IPT * DP)).astype(np.float32)

        # bias: [t, 128, NPC*DP] f32 (with Q15 offset correction baked in)
        b_c = bq[:, lo:lo + ND].reshape(STEPS, 8, NPC)
        bx = np.repeat(b_c[:, :, None, :], 16, axis=2)
        bx = np.repeat(bx[..., None], DP, axis=-1)
        bias_host = np.ascontiguousarray(
            bx.reshape(STEPS, 128, NPC * DP)).astype(np.float32)

        # xin: [128, NPC*DP] f32: part 16q+j, col n*4+kk = x[4j+kk, node]
        xc = x[:, lo:lo + ND].reshape(16, DP, 8, NPC)  # [j, kk, q, n]
        xin = np.ascontiguousarray(
            xc.transpose(2, 0, 3, 1).reshape(128, NPC * DP))

        def expand_node_vec(v):
            vc = v[lo:lo + ND].reshape(8, NPC)
            vx = np.repeat(vc[:, None, :], 16, axis=1)
            vx = np.repeat(vx[..., None], DP, axis=-1)
            return np.ascontiguousarray(
                vx.reshape(128, NPC * DP).astype(np.float32))

        in_maps.append({
            "idx": idx,
            "w": w_host,
            "bias": bias_host,
            "xin": xin,
            "winp": expand_node_vec(w_in),
            "b0p": expand_node_vec(biases[0]),
        })
    return in_maps


def _unpack_out(results):
    out = np.empty((B, N), np.float32)
    for c in range(NCORES):
        o = results[c]["out"].reshape(8, 16, NPC, DP)  # [q, j, n, kk]
        out[:, ND * c:ND * (c + 1)] = (
            o.transpose(1, 3, 0, 2).reshape(B, ND))
    return out


def kernel(x, w_in, edge_w, biases, edge_src):
    from concourse.bass_utils import run_bass_kernel_spmd

    if "nc" not in _CACHE:
        _CACHE["nc"] = _build_kernel()
    nc = _CACHE["nc"]

    in_maps = _prep_static(edge_src, edge_w, biases, w_in, x)
    res = run_bass_kernel_spmd(nc, in_maps, list(range(NCORES)))
    return _unpack_out(res.results)
